# revision 21
# baseline (speedup 1.0000x reference)
"""Trainium2 Bass kernel for nn_DecoderBlock_82420422410637.

Math (the reference's FeedForward block is dead code -- the final ternary
`... if False else x + full(0.01)*0` reduces to `x`):

    h   = layernorm(x, w1, b1)
    qkv = h @ qkv_w ;  q,k,v per head (H=12, D=64)
    S   = q @ k^T * D^-0.5 ; P = softmax(S)
    v_content = P @ v
    v_pos     = segment-mean of v over sector_ids, gathered back
    out_h = g*v_pos + (1-g)*v_content ,  g = sigmoid(gate_logit_h)
    attn  = concat(out_h) @ proj_w + proj_b
    out   = x + ls1_gamma * attn

Sharding: 8 cores = 4 batches x 2 head-groups (6 heads each).  The host
applies layernorm (xn) and the residual x + ls1*proj_b; each core
returns its heads' bf16 partial of ls1 * (heads @ proj_w).

Per-core phases:
  A qkT = Wqk^T @ xn^T    fp8 DoubleRow (contraction 256/pass), weights
      host-scaled x64 (fp8e4m3 can't hold 0.02-scale weights), drains
      rescale by 1/64 into bf16.
  B v token-major          fp8 DoubleRow, same scaling.
  C positional: segsum -> gsc -> transpose -> Z = m1^T @ pw (bf16)
  D attention per pair (2 heads, row-split QK^T), software-pipelined:
      per kc step the PE runs [PV_j1(kc-1), S-pair(kc+1), PV_j0(kc)]
      while ACT exps head j0 (exact) and DVE exps head j1 via the
      Schraudolph int16 bit trick:
          bf16_bits(int16(S*A + B)) ~= exp(S*scale), |rel err| < 3.6%
      Pair 0/1 normalisation: gpsimd multiply (single Q7 library) with
      the reciprocal row replicated by a stride-0-free-dim DMA; results
      land as the fp8 DoubleRow stationary for phase E.  Pair 2 (the
      latency-critical one) broadcasts via a rank-1 PE matmul
      (64/denom = ones64^T @ recip-row) into a freed PSUM slot and
      multiplies on DVE.
  E proj: per token chunk one bf16 matmul (oht@Z), one fp8 DoubleRow
      matmul (pairs 0+1), and deferred-by-3 bf16 matmuls for pair 2;
      4 rotating PSUM accumulators; drain rescales by 1/64.

PSUM budget = 8 banks: psS 2x[128,1024] + psV0/psV1 1x[65,1024] each.
"""

import os
import sys
from contextlib import ExitStack

import numpy as np

for _p in ("/opt/trn_rl_repo", "/root/.axon_site/_ro/trn_rl_repo"):
    if os.path.isdir(_p) and _p not in sys.path:
        sys.path.append(_p)

import ml_dtypes  # noqa: E402
import concourse.bass as bass  # noqa: E402
import concourse.mybir as mybir  # noqa: E402
import concourse.tile as tile  # noqa: E402
from concourse import bacc, bass_utils  # noqa: E402

F32 = mybir.dt.float32
BF16 = mybir.dt.bfloat16
F8 = mybir.dt.float8e4
I16 = mybir.dt.int16
AF = mybir.ActivationFunctionType
ALU = mybir.AluOpType
DR = mybir.MatmulPerfMode.DoubleRow

B, N, C, H, D, S = 4, 1024, 768, 12, 64, 11
HL = H // 2          # heads per core (6)
CK = C // 128        # 6 contraction chunks (bf16) / 3 (DoubleRow)
DK = C // 256        # 3 DoubleRow contraction chunks
TC = N // 128        # 8 token chunks
QC = N // 512        # 2 query chunks
PAIRS = HL // 2      # 3 head pairs per core
EPS = 1e-5
SCALE = D ** -0.5
WS = 64.0            # fp8 weight prescale
# Schraudolph: bf16_bits(int16(x*A_SCH + B_SCH)) ~= exp(x*SCALE)
A_SCH = (2.0 ** 7 / float(np.log(2.0))) * SCALE
B_SCH = 127.0 * 128.0 - 5.5

_CACHED = {}


def _build_program():
    nc = bacc.Bacc("TRN2", target_bir_lowering=False, debug=False)

    xnT8 = nc.dram_tensor("xnT8", [DK * 128, 2 * N], F8, kind="ExternalInput")
    qkw8 = nc.dram_tensor("qkw8", [DK * 128, 2 * 2 * HL * D], F8,
                          kind="ExternalInput")
    vw8 = nc.dram_tensor("vw8", [DK * 128, 2 * HL * D], F8,
                         kind="ExternalInput")
    pw8 = nc.dram_tensor("pw8", [128, 2 * C], F8, kind="ExternalInput")
    pw = nc.dram_tensor("pw", [HL * D, C], BF16, kind="ExternalInput")
    oh = nc.dram_tensor("oh", [N, S], BF16, kind="ExternalInput")
    oht = nc.dram_tensor("oht", [S, N], BF16, kind="ExternalInput")
    gscf = nc.dram_tensor("gscf", [S, HL * D], F32, kind="ExternalInput")
    vcol = nc.dram_tensor("vcol", [128, HL], BF16, kind="ExternalInput")
    out = nc.dram_tensor("out", [N, C], BF16, kind="ExternalOutput")

    def re2(ap):
        return ap.rearrange("p (h n) -> p h n", h=2)

    with tile.TileContext(nc) as tc:
        with ExitStack() as ctx:
            cpool = ctx.enter_context(tc.tile_pool(name="consts", bufs=1))
            qkpool = ctx.enter_context(tc.tile_pool(name="qkt", bufs=1))
            vpool = ctx.enter_context(tc.tile_pool(name="v", bufs=1))
            e0pool = ctx.enter_context(tc.tile_pool(name="e0", bufs=3))
            e1pool = ctx.enter_context(tc.tile_pool(name="e1", bufs=3))
            vcpool = ctx.enter_context(tc.tile_pool(name="vcat", bufs=1))
            dpool = ctx.enter_context(tc.tile_pool(name="drain", bufs=2))
            rpool = ctx.enter_context(tc.tile_pool(name="rr", bufs=2))
            mpool = ctx.enter_context(tc.tile_pool(name="m1", bufs=1))
            opool = ctx.enter_context(tc.tile_pool(name="out", bufs=3))
            # PSUM: 4 + 2 + 2 banks
            psS = ctx.enter_context(tc.tile_pool(name="psS", bufs=2, space="PSUM"))
            psV0 = ctx.enter_context(tc.tile_pool(name="psV0", bufs=1, space="PSUM"))
            psV1 = ctx.enter_context(tc.tile_pool(name="psV1", bufs=1, space="PSUM"))

            # ---- loads (first A-chunk inputs first) ----
            xnT8_t = []
            qkw8_t = []
            for k in range(DK):
                t = cpool.tile([128, 2 * N], F8, tag=f"xnT8{k}")
                nc.sync.dma_start(t[:], xnT8.ap()[k * 128:(k + 1) * 128, :])
                xnT8_t.append(t)
                t = cpool.tile([128, 2 * 2 * HL * D], F8, tag=f"qkw8{k}")
                nc.gpsimd.dma_start(t[:], qkw8.ap()[k * 128:(k + 1) * 128, :])
                qkw8_t.append(t)
            vw8_t = []
            for k in range(DK):
                t = cpool.tile([128, 2 * HL * D], F8, tag=f"vw8{k}")
                nc.gpsimd.dma_start(t[:], vw8.ap()[k * 128:(k + 1) * 128, :])
                vw8_t.append(t)
            pw8_t = cpool.tile([128, 2 * C], F8, tag="pw8")
            nc.gpsimd.dma_start(pw8_t[:], pw8.ap()[:, :])
            pw_t = []
            for k in range(PAIRS):
                t = cpool.tile([128, C], BF16, tag=f"pw{k}")
                nc.sync.dma_start(t[:], pw.ap()[k * 128:(k + 1) * 128, :])
                pw_t.append(t)
            oh_t = []
            for kc in range(TC):
                t = cpool.tile([128, S], BF16, tag=f"oh{kc}")
                nc.gpsimd.dma_start(t[:], oh.ap()[kc * 128:(kc + 1) * 128, :])
                oh_t.append(t)
            oht_t = cpool.tile([S, N], BF16, tag="oht")
            nc.gpsimd.dma_start(oht_t[:], oht.ap()[:, :])
            gscf_t = cpool.tile([S, HL * D], F32, tag="gscf")
            nc.gpsimd.dma_start(gscf_t[:], gscf.ap()[:, :])
            vcol_t = cpool.tile([128, HL], BF16, tag="vcol")
            nc.gpsimd.dma_start(vcol_t[:], vcol.ap()[:, :])
            ident_t = cpool.tile([128, 128], BF16, tag="ident")
            from concourse.masks import make_identity
            make_identity(nc, ident_t[:])
            ones64 = cpool.tile([1, 64], BF16, tag="ones64")
            nc.gpsimd.memset(ones64[:], WS)

            # warm the ACT exp table set and the gpsimd tensor_tensor Q7
            # library early (each costs ~2.7/6us; overlap DMAs / phase A)
            dum = cpool.tile([1, 8], F32, tag="dum")
            nc.gpsimd.memset(dum[:], 0.0)
            dum2 = cpool.tile([1, 8], F32, tag="dum2")
            nc.scalar.activation(dum2[:], dum[:], AF.Exp)

            # ---- A: qkT[m] = (qkw chunk m)^T @ xnT, fp8 DoubleRow ----
            qkT = [qkpool.tile([128, N], BF16, tag=f"qkT{m}", name=f"qkT{m}")
                   for m in range(CK)]
            for i, m in enumerate((0, 3, 1, 4, 2, 5)):
                ps = psS.tile([128, N], F32, tag="s")
                for n_i in range(QC):
                    for k in range(DK):
                        nc.tensor.matmul(
                            ps[:, n_i * 512:(n_i + 1) * 512],
                            re2(qkw8_t[k][:])[:, :, m * 128:(m + 1) * 128],
                            re2(xnT8_t[k][:])[:, :, n_i * 512:(n_i + 1) * 512],
                            start=(k == 0), stop=(k == DK - 1),
                            perf_mode=DR,
                        )
                if i % 2 == 0:
                    nc.scalar.mul(qkT[m][:], ps[:], 1.0 / WS)
                else:
                    nc.vector.tensor_scalar(qkT[m][:], ps[:], 1.0 / WS, None,
                                            ALU.mult)

            # ---- B: v token-major, 65-col head blocks (col 64 = 1/(1-g)) ----
            vt = [vpool.tile([128, HL * (D + 1)], BF16, tag=f"v{kc}", name=f"v{kc}")
                  for kc in range(TC)]
            for kc in range(TC):
                nc.gpsimd.dma_start(
                    vt[kc][:].rearrange("p (h c) -> p h c", c=D + 1)[:, :, D:D + 1],
                    vcol_t[:],
                )
            for kc in range(TC):
                pool = psV0 if kc % 2 == 0 else psV1
                psv = pool.tile([128, HL * D], F32,
                                tag="v0" if kc % 2 == 0 else "v1")
                for k in range(DK):
                    nc.tensor.matmul(
                        psv[:],
                        re2(xnT8_t[k][:])[:, :, kc * 128:(kc + 1) * 128],
                        re2(vw8_t[k][:]),
                        start=(k == 0), stop=(k == DK - 1),
                        perf_mode=DR,
                    )
                nc.vector.tensor_scalar(
                    vt[kc][:].rearrange("p (h c) -> p h c", c=D + 1)[:, :, 0:D],
                    psv[:].rearrange("p (h c) -> p h c", c=D),
                    1.0 / WS, None, ALU.mult,
                )

            # ---- C: positional branch -> Z (11 x 768), x64 scaled ----
            psm = psV0.tile([S, HL * D], F32, tag="v0")
            for kc in range(TC):
                nc.tensor.matmul(
                    psm[0:S, :],
                    oh_t[kc][:, 0:S],
                    vt[kc][:].rearrange("p (h c) -> p h c", c=D + 1)[:, :, 0:D],
                    start=(kc == 0), stop=(kc == TC - 1),
                )
            m1n = mpool.tile([S, HL * D], BF16, tag="m1n")
            nc.vector.tensor_tensor(m1n[0:S, :], psm[0:S, :], gscf_t[0:S, :],
                                    ALU.mult)
            pst = psV1.tile([128, 3 * 16], BF16, tag="v1")
            for k3 in range(PAIRS):
                nc.tensor.transpose(
                    pst[:, k3 * 16:k3 * 16 + S],
                    m1n[0:S, k3 * 128:(k3 + 1) * 128],
                    ident_t[0:S, 0:S],
                )
            m1T = mpool.tile([128, 3 * 16], BF16, tag="m1T")
            nc.vector.tensor_copy(m1T[:], pst[:])
            psz = psS.tile([S, C], F32, tag="s")
            for (c0, c1) in ((0, 512), (512, C)):
                for k3 in range(PAIRS):
                    nc.tensor.matmul(psz[0:S, c0:c1], m1T[:, k3 * 16:k3 * 16 + S],
                                     pw_t[k3][:, c0:c1],
                                     start=(k3 == 0), stop=(k3 == PAIRS - 1))
            zb = mpool.tile([S, C], BF16, tag="zb")
            nc.vector.tensor_scalar(zb[0:S, :], psz[0:S, :], WS, None, ALU.mult)

            # ---- D: attention, software-pipelined ----
            # pairs 0/1 normalise into the fp8 DoubleRow stationary
            # vcat01[(h=pair), j*64+d, token]; pair 2 into bf16 vcat2
            # scaled x64 via the PE broadcast.
            vcat01 = vcpool.tile([128, 2 * N], F8, tag="vc01")
            vcat2 = vcpool.tile([128, N], BF16, tag="vc2")

            def s_pair(p, kc):
                """QK^T for both heads of pair p, key chunk kc (row-split)."""
                t0 = psS.tile([128, N], F32, tag="s", name=f"sS{p}_{kc}_0")
                t1 = psS.tile([128, N], F32, tag="s", name=f"sS{p}_{kc}_1")
                for qc in range(QC):
                    for j, ps in ((0, t0), (1, t1)):
                        off = j * 64
                        nc.tensor.matmul(
                            ps[:, qc * 512:(qc + 1) * 512],
                            qkT[3 + p][off:off + 64, kc * 128:(kc + 1) * 128],
                            qkT[p][off:off + 64, qc * 512:(qc + 1) * 512],
                            start=True, stop=True,
                            tile_position=(off, 0),
                        )
                return t0, t1

            def pv(p, j, kc, e_ap, psv):
                hidx = 2 * p + j
                for qc in range(QC):
                    nc.tensor.matmul(
                        psv[0:D + 1, qc * 512:(qc + 1) * 512],
                        vt[kc][:, hidx * (D + 1):(hidx + 1) * (D + 1)],
                        e_ap[:, qc * 512:(qc + 1) * 512],
                        start=(kc == 0), stop=(kc == TC - 1),
                    )

            def drain01(p, j, psv):
                """Pairs 0/1: reciprocal row -> gpsimd partition_broadcast
                (only Q7 op in the kernel, so a single library; its ~4us
                drain latency hides under the next pair) -> DVE multiply.
                A stride-0 DMA broadcast would be faster but its completion
                semaphore undercounts -> race -> intermittent NaN."""
                vcp = dpool.tile([65, N], BF16, tag=f"vcp{j}", name=f"vcp{p}_{j}")
                nc.scalar.copy(vcp[0:65, :], psv[0:65, :])
                packed = rpool.tile([64, 16], BF16, tag=f"packed{j}",
                                    name=f"packed{p}_{j}")
                nc.sync.dma_start(packed[:], vcp[64:65, :])
                rec = rpool.tile([64, 16], BF16, tag=f"rec{j}", name=f"rec{p}_{j}")
                with nc.allow_low_precision(reason="softmax denom, ample tol"):
                    nc.vector.reciprocal(rec[:], packed[:])
                rrt = rpool.tile([1, N], BF16, tag=f"rrt{j}", name=f"rrt{p}_{j}")
                nc.sync.dma_start(rrt[0:1, :], rec[:])
                rbc = dpool.tile([64, N], BF16, tag=f"rbc{j}", name=f"rbc{p}_{j}")
                nc.gpsimd.partition_broadcast(rbc[:], rrt[0:1, :])
                nc.vector.tensor_tensor(
                    re2(vcat01[:])[j * 64:(j + 1) * 64, p, :],
                    vcp[0:64, :], rbc[:], ALU.mult,
                )

            def drain2_pre(j, psv, rbc_pool, rbc_tag):
                """Pair 2, stage 1: copy + reciprocal row (through rrt).
                Allocates the broadcast PSUM tile now so later pool users
                order correctly; the PE matmul comes in drain2_mm."""
                vcp = dpool.tile([65, N], BF16, tag=f"vcp{j}", name=f"vcp2_{j}")
                if j == 0:
                    nc.scalar.copy(vcp[0:65, :], psv[0:65, :])
                else:
                    nc.vector.tensor_copy(vcp[0:65, :], psv[0:65, :])
                packed = rpool.tile([64, 16], BF16, tag=f"packed{j}",
                                    name=f"packed2_{j}")
                nc.sync.dma_start(packed[:], vcp[64:65, :])
                rec = rpool.tile([64, 16], BF16, tag=f"rec{j}", name=f"rec2_{j}")
                with nc.allow_low_precision(reason="softmax denom, ample tol"):
                    nc.vector.reciprocal(rec[:], packed[:])
                rrt = rpool.tile([1, N], BF16, tag=f"rrt{j}", name=f"rrt2_{j}")
                nc.sync.dma_start(rrt[0:1, :], rec[:])
                rbc_ps = rbc_pool.tile([64, N], F32, tag=rbc_tag,
                                       name=f"rbcps{j}")
                return vcp, rrt, rbc_ps

            def drain2_mm(j, vcp, rrt, rbc_ps):
                """Pair 2, stage 2: rank-1 PE broadcast (64/denom) then
                DVE multiply -> bf16 vcat2 (x64)."""
                for qc in range(QC):
                    nc.tensor.matmul(
                        rbc_ps[0:64, qc * 512:(qc + 1) * 512],
                        ones64[0:1, :],
                        rrt[0:1, qc * 512:(qc + 1) * 512],
                        start=True, stop=True,
                    )
                nc.vector.tensor_tensor(
                    vcat2[j * 64:(j + 1) * 64, :],
                    vcp[0:64, :], rbc_ps[0:64, :], ALU.mult,
                )

            pend = None
            for p in range(PAIRS):
                sA = s_pair(p, 0)
                psv0 = psV0.tile([D + 1, N], F32, tag="v0", name=f"psV0_{p}")
                psv1 = psV1.tile([D + 1, N], F32, tag="v1", name=f"psV1_{p}")
                e1_prev = None
                for kc in range(TC):
                    # exp j0 on ACT (exact), j1 on DVE (Schraudolph)
                    e0 = e0pool.tile([128, N], BF16, tag="e0")
                    nc.scalar.activation(e0[:], sA[0][:], AF.Exp, scale=SCALE)
                    e1 = e1pool.tile([128, N], I16, tag="e1")
                    with nc.allow_low_precision(reason="schraudolph exp"):
                        nc.vector.tensor_scalar(
                            e1[:], sA[1][:], A_SCH, B_SCH, ALU.mult, ALU.add
                        )
                    # PE block: [PV_j1(lag)] [S(kc+1)] [PV_j0(kc)]
                    # (last step: PV_j0 first so psv0 finishes earlier and
                    # its drain chain starts sooner)
                    if pend is not None:
                        pp, ppsv, pe1 = pend
                        pv(pp, 1, TC - 1, pe1[:].bitcast(BF16), ppsv)
                        drain01(pp, 1, ppsv)
                        pend = None
                    elif kc == TC - 1:
                        pv(p, 0, kc, e0[:], psv0)
                        pv(p, 1, kc - 1, e1_prev[:].bitcast(BF16), psv1)
                        e1_prev = e1
                        continue
                    elif kc > 0:
                        pv(p, 1, kc - 1, e1_prev[:].bitcast(BF16), psv1)
                    if kc < TC - 1:
                        sA = s_pair(p, kc + 1)
                    pv(p, 0, kc, e0[:], psv0)
                    e1_prev = e1
                if p < PAIRS - 1:
                    drain01(p, 0, psv0)
                    pend = (p, psv1, e1_prev)
                else:
                    pv(p, 1, TC - 1, e1_prev[:].bitcast(BF16), psv1)
                    pair2_psvs = (psv0, psv1)

            # ---- E: proj (+Z); pair-2 matmuls deferred 3 slots ----
            # (pair-2 drains are emitted after two e_partial blocks so the
            # PE chews on them while the reciprocal chain runs)
            po_tiles = {}

            def e_partial(t_i):
                pool, tag = ((psS, "s"), (psS, "s"), (psV0, "v0"),
                             (psV1, "v1"))[t_i % 4]
                po = pool.tile([128, C], F32, tag=tag, name=f"po{t_i}")
                po_tiles[t_i] = po
                for (c0, c1) in ((0, 512), (512, C)):
                    nc.tensor.matmul(
                        po[:, c0:c1],
                        oht_t[0:S, t_i * 128:(t_i + 1) * 128],
                        zb[0:S, c0:c1],
                        start=True, stop=False,
                    )
                    nc.tensor.matmul(
                        po[:, c0:c1],
                        re2(vcat01[:])[:, :, t_i * 128:(t_i + 1) * 128],
                        re2(pw8_t[:])[:, :, c0:c1],
                        start=False, stop=False,
                        perf_mode=DR,
                    )

            def e_final(t_i):
                po = po_tiles.pop(t_i)
                for (c0, c1) in ((0, 512), (512, C)):
                    nc.tensor.matmul(
                        po[:, c0:c1],
                        vcat2[:, t_i * 128:(t_i + 1) * 128],
                        pw_t[2][:, c0:c1],
                        start=False, stop=True,
                    )
                ot = opool.tile([128, C], BF16, tag="ot")
                if t_i % 2 == 0:
                    nc.scalar.mul(ot[:], po[:], 1.0 / WS)
                else:
                    nc.vector.tensor_scalar(ot[:], po[:], 1.0 / WS, None,
                                            ALU.mult)
                nc.sync.dma_start(out.ap()[t_i * 128:(t_i + 1) * 128, :], ot[:])

            e_partial(0)
            e_partial(1)
            pre0 = drain2_pre(0, pair2_psvs[0], psV0, "v0")
            pre1 = drain2_pre(1, pair2_psvs[1], psV1, "v1")
            drain2_mm(0, *pre0)
            drain2_mm(1, *pre1)
            e_partial(2)
            e_partial(3)
            for t_i in range(4, TC):
                e_final(t_i - 4)
                e_partial(t_i)
            for t_i in range(4, TC):
                e_final(t_i)

    nc.compile()
    return nc


def _sigmoid(x):
    return 1.0 / (1.0 + np.exp(-x))


def _pack_dr(m):
    """[R, C] -> [R/2, 2C]: row 128k+p holds rows 256k+p | 256k+128+p."""
    r, c = m.shape
    return (m.reshape(r // 256, 2, 128, c).transpose(0, 2, 1, 3)
            .reshape(r // 2, 2 * c))


def _prep_core_inputs(cid, x, sector_ids, qkv_w, proj_w, gate_logit,
                      norm1_w, norm1_b, ls1_gamma):
    b, hg = cid // 2, cid % 2
    bf = ml_dtypes.bfloat16
    f8 = ml_dtypes.float8_e4m3
    h0 = hg * HL

    xb = x[b].astype(np.float64)
    mu = xb.mean(axis=-1, keepdims=True)
    var = xb.var(axis=-1, keepdims=True)
    xn = ((xb - mu) / np.sqrt(var + EPS)) * norm1_w + norm1_b  # (N, C)

    wq = qkv_w[:, h0 * D:(h0 + HL) * D]
    wk = qkv_w[:, C + h0 * D:C + (h0 + HL) * D]
    wv = qkv_w[:, 2 * C + h0 * D:2 * C + (h0 + HL) * D]
    qkw = np.concatenate([wq, wk], axis=1)
    pw_eff = proj_w[h0 * D:(h0 + HL) * D, :] * ls1_gamma[None, :]  # (384,768)

    g = _sigmoid(gate_logit.astype(np.float64))[h0:h0 + HL]  # (6,)

    onehot = np.zeros((N, S), np.float32)
    onehot[np.arange(N), sector_ids] = 1.0
    counts = onehot.sum(axis=0)                              # (11,)
    gsc = (g[None, :] / np.maximum(counts, 1.0)[:, None])    # (11, 6)
    gscf = np.repeat(gsc, D, axis=1).astype(np.float32)      # (11, 384)
    vcol = np.broadcast_to((1.0 / (1.0 - g))[None, :], (128, HL))

    return {
        "xnT8": np.ascontiguousarray(_pack_dr(xn.T).astype(f8)),
        "qkw8": np.ascontiguousarray(_pack_dr(qkw * WS).astype(f8)),
        "vw8": np.ascontiguousarray(_pack_dr(wv * WS).astype(f8)),
        "pw8": np.ascontiguousarray(
            (pw_eff[0:256] * WS).reshape(2, 128, C).transpose(1, 0, 2)
            .reshape(128, 2 * C).astype(f8)),
        "pw": np.ascontiguousarray(pw_eff.astype(bf)),
        "oh": np.ascontiguousarray(onehot.astype(bf)),
        "oht": np.ascontiguousarray(onehot.T.astype(bf)),
        "gscf": gscf,
        "vcol": np.ascontiguousarray(vcol.astype(bf)),
    }


def kernel(x, sector_ids, qkv_w, proj_w, proj_b, gate_logit,
           norm1_w, norm1_b, ls1_gamma, norm2_w, norm2_b,
           ff_w1, ff_b1, ff_w2, ff_b2, _want_trace=False):
    x = np.asarray(x, np.float32)
    sector_ids = np.asarray(sector_ids).astype(np.int64)
    args = [np.asarray(a, np.float32) for a in
            (qkv_w, proj_w, gate_logit, norm1_w, norm1_b, ls1_gamma)]

    in_maps = [_prep_core_inputs(cid, x, sector_ids, *args) for cid in range(8)]

    if "prog" not in _CACHED:
        _CACHED["prog"] = _build_program()
    nc = _CACHED["prog"]

    import concourse.mybir as _mb
    expected = set()
    for alloc in nc.m.functions[0].allocations:
        if isinstance(alloc, _mb.MemoryLocationSet) and alloc.kind == "ExternalInput":
            expected.add(alloc.memorylocations[0].name)
    in_maps = [{k: v for k, v in m.items() if k in expected} for m in in_maps]

    res = bass_utils.run_bass_kernel_spmd(
        nc, in_maps, core_ids=list(range(8)), trace=_want_trace
    )
    if _want_trace:
        _CACHED["last_result"] = res

    outs = [r["out"].astype(np.float32) for r in res.results]
    proj_b = np.asarray(proj_b, np.float32)
    ls1 = np.asarray(ls1_gamma, np.float32)
    full = np.empty((B, N, C), np.float32)
    for b in range(B):
        full[b] = x[b] + outs[2 * b] + outs[2 * b + 1] + (ls1 * proj_b)[None, :]
    return full


# revision 33
# speedup vs baseline: 1.0962x; 1.0962x over previous
"""Trainium2 Bass kernel for nn_DecoderBlock_82420422410637.

Math (the reference's FeedForward block is dead code -- the final ternary
`... if False else x + full(0.01)*0` reduces to `x`):

    h   = layernorm(x, w1, b1)
    qkv = h @ qkv_w ;  q,k,v per head (H=12, D=64)
    S   = q @ k^T * D^-0.5 ; P = softmax(S)
    v_content = P @ v
    v_pos     = segment-mean of v over sector_ids, gathered back
    out_h = g*v_pos + (1-g)*v_content ,  g = sigmoid(gate_logit_h)
    attn  = concat(out_h) @ proj_w + proj_b
    out   = x + ls1_gamma * attn

Sharding: 8 cores = 4 batches x 2 head-groups (6 heads each).  The host
applies layernorm (xn) and the residual x + ls1*proj_b; each core
returns its heads' bf16 partial of ls1 * (heads @ proj_w).

Per-core phases:
  A qkT = Wqk^T @ xn^T    fp8 DoubleRow (contraction 256/pass), weights
      host-scaled x64 (fp8e4m3 can't hold 0.02-scale weights), drains
      rescale by 1/64 into bf16.
  B v token-major          fp8 DoubleRow, same scaling.
  C positional: segsum -> gsc -> transpose -> Z = m1^T @ pw (bf16)
  D attention per pair (2 heads, row-split QK^T), software-pipelined:
      per kc step the PE runs [PV_j1(kc-1), S-pair(kc+1), PV_j0(kc)]
      while ACT exps head j0 (exact) and DVE exps head j1 via the
      Schraudolph int16 bit trick:
          bf16_bits(int16(S*A + B)) ~= exp(S*scale), |rel err| < 3.6%
      Pair 0/1 normalisation: gpsimd multiply (single Q7 library) with
      the reciprocal row replicated by a stride-0-free-dim DMA; results
      land as the fp8 DoubleRow stationary for phase E.  Pair 2 (the
      latency-critical one) broadcasts via a rank-1 PE matmul
      (64/denom = ones64^T @ recip-row) into a freed PSUM slot and
      multiplies on DVE.
  E proj: per token chunk one bf16 matmul (oht@Z), one fp8 DoubleRow
      matmul (pairs 0+1), and deferred-by-3 bf16 matmuls for pair 2;
      4 rotating PSUM accumulators; drain rescales by 1/64.

PSUM budget = 8 banks: psS 2x[128,1024] + psV0/psV1 1x[65,1024] each.
"""

import os
import sys
from contextlib import ExitStack

import numpy as np

for _p in ("/opt/trn_rl_repo", "/root/.axon_site/_ro/trn_rl_repo"):
    if os.path.isdir(_p) and _p not in sys.path:
        sys.path.append(_p)

import ml_dtypes  # noqa: E402
import concourse.bass as bass  # noqa: E402
import concourse.mybir as mybir  # noqa: E402
import concourse.tile as tile  # noqa: E402
from concourse import bacc, bass_utils  # noqa: E402

F32 = mybir.dt.float32
BF16 = mybir.dt.bfloat16
F8 = mybir.dt.float8e4
U8 = mybir.dt.uint8
AF = mybir.ActivationFunctionType
ALU = mybir.AluOpType
DR = mybir.MatmulPerfMode.DoubleRow

B, N, C, H, D, S = 4, 1024, 768, 12, 64, 11
HL = H // 2          # heads per core (6)
CK = C // 128        # 6 contraction chunks (bf16) / 3 (DoubleRow)
DK = C // 256        # 3 DoubleRow contraction chunks
TC = N // 128        # 8 token chunks
QC = N // 512        # 2 query chunks
PAIRS = HL // 2      # 3 head pairs per core
EPS = 1e-5
SCALE = D ** -0.5
WS = 64.0            # fp8 weight prescale
# Schraudolph: f8e4m3_bits(uint8(x*A_SCH + B_SCH)) ~= exp(x*SCALE),
# |rel err| < 7.5% on logits in [-4, 5]; uint8 saturation flushes
# impossible ultra-negative logits to +0 instead of NaN.
A_SCH = (2.0 ** 3 / float(np.log(2.0))) * SCALE
B_SCH = 7.0 * 8.0 + 0.15

_CACHED = {}


def _build_program():
    nc = bacc.Bacc("TRN2", target_bir_lowering=False, debug=False)

    xnT8 = nc.dram_tensor("xnT8", [DK * 128, 2 * N], F8, kind="ExternalInput")
    qkw8 = nc.dram_tensor("qkw8", [DK * 128, 2 * 2 * HL * D], F8,
                          kind="ExternalInput")
    vw8 = nc.dram_tensor("vw8", [DK * 128, 2 * HL * D], F8,
                         kind="ExternalInput")
    pw8 = nc.dram_tensor("pw8", [128, 2 * C], F8, kind="ExternalInput")
    pw = nc.dram_tensor("pw", [HL * D, C], BF16, kind="ExternalInput")
    oh = nc.dram_tensor("oh", [N, S], F8, kind="ExternalInput")
    oht = nc.dram_tensor("oht", [S, N], BF16, kind="ExternalInput")
    gscf = nc.dram_tensor("gscf", [S, HL * D], F32, kind="ExternalInput")
    vcol = nc.dram_tensor("vcol", [128, HL], F8, kind="ExternalInput")
    out = nc.dram_tensor("out", [N, C], BF16, kind="ExternalOutput")

    def re2(ap):
        return ap.rearrange("p (h n) -> p h n", h=2)

    with tile.TileContext(nc) as tc:
        with ExitStack() as ctx:
            cpool = ctx.enter_context(tc.tile_pool(name="consts", bufs=1))
            qkpool = ctx.enter_context(tc.tile_pool(name="qkt", bufs=1))
            vpool = ctx.enter_context(tc.tile_pool(name="v", bufs=1))
            e0pool = ctx.enter_context(tc.tile_pool(name="e0", bufs=3))
            e1pool = ctx.enter_context(tc.tile_pool(name="e1", bufs=3))
            vcpool = ctx.enter_context(tc.tile_pool(name="vcat", bufs=1))
            dpool = ctx.enter_context(tc.tile_pool(name="drain", bufs=2))
            rpool = ctx.enter_context(tc.tile_pool(name="rr", bufs=2))
            mpool = ctx.enter_context(tc.tile_pool(name="m1", bufs=1))
            opool = ctx.enter_context(tc.tile_pool(name="out", bufs=3))
            # PSUM: 4 + 2 + 2 banks
            psS = ctx.enter_context(tc.tile_pool(name="psS", bufs=2, space="PSUM"))
            psV0 = ctx.enter_context(tc.tile_pool(name="psV0", bufs=1, space="PSUM"))
            psV1 = ctx.enter_context(tc.tile_pool(name="psV1", bufs=1, space="PSUM"))

            # ---- loads (first A-chunk inputs first) ----
            xnT8_t = []
            qkw8_t = []
            for k in range(DK):
                t = cpool.tile([128, 2 * N], F8, tag=f"xnT8{k}")
                nc.sync.dma_start(t[:], xnT8.ap()[k * 128:(k + 1) * 128, :])
                xnT8_t.append(t)
                t = cpool.tile([128, 2 * 2 * HL * D], F8, tag=f"qkw8{k}")
                nc.gpsimd.dma_start(t[:], qkw8.ap()[k * 128:(k + 1) * 128, :])
                qkw8_t.append(t)
            vw8_t = []
            for k in range(DK):
                t = cpool.tile([128, 2 * HL * D], F8, tag=f"vw8{k}")
                nc.gpsimd.dma_start(t[:], vw8.ap()[k * 128:(k + 1) * 128, :])
                vw8_t.append(t)
            pw8_t = cpool.tile([128, 2 * C], F8, tag="pw8")
            nc.gpsimd.dma_start(pw8_t[:], pw8.ap()[:, :])
            pw_t = []
            for k in range(PAIRS):
                t = cpool.tile([128, C], BF16, tag=f"pw{k}")
                nc.sync.dma_start(t[:], pw.ap()[k * 128:(k + 1) * 128, :])
                pw_t.append(t)
            oh_t = []
            for kc in range(TC):
                t = cpool.tile([128, S], F8, tag=f"oh{kc}")
                nc.gpsimd.dma_start(t[:], oh.ap()[kc * 128:(kc + 1) * 128, :])
                oh_t.append(t)
            oht_t = cpool.tile([S, N], BF16, tag="oht")
            nc.gpsimd.dma_start(oht_t[:], oht.ap()[:, :])
            gscf_t = cpool.tile([S, HL * D], F32, tag="gscf")
            nc.gpsimd.dma_start(gscf_t[:], gscf.ap()[:, :])
            vcol_t = cpool.tile([128, HL], F8, tag="vcol")
            nc.gpsimd.dma_start(vcol_t[:], vcol.ap()[:, :])
            ident_t = cpool.tile([128, 128], BF16, tag="ident")
            from concourse.masks import make_identity
            make_identity(nc, ident_t[:])
            ones64 = cpool.tile([1, 64], BF16, tag="ones64")
            nc.gpsimd.memset(ones64[:], WS)

            # warm the ACT exp table set and the gpsimd tensor_tensor Q7
            # library early (each costs ~2.7/6us; overlap DMAs / phase A)
            dum = cpool.tile([1, 8], F32, tag="dum")
            nc.gpsimd.memset(dum[:], 0.0)
            dum2 = cpool.tile([1, 8], F32, tag="dum2")
            nc.scalar.activation(dum2[:], dum[:], AF.Exp)

            # ---- A: qkT[m] = (qkw chunk m)^T @ xnT, fp8 DoubleRow ----
            qkT = [qkpool.tile([128, N], BF16, tag=f"qkT{m}", name=f"qkT{m}")
                   for m in range(CK)]
            for i, m in enumerate((0, 3, 1, 4, 2, 5)):
                ps = psS.tile([128, N], F32, tag="s")
                for n_i in range(QC):
                    for k in range(DK):
                        nc.tensor.matmul(
                            ps[:, n_i * 512:(n_i + 1) * 512],
                            re2(qkw8_t[k][:])[:, :, m * 128:(m + 1) * 128],
                            re2(xnT8_t[k][:])[:, :, n_i * 512:(n_i + 1) * 512],
                            start=(k == 0), stop=(k == DK - 1),
                            perf_mode=DR,
                        )
                if i % 2 == 0:
                    nc.scalar.mul(qkT[m][:], ps[:], 1.0 / WS)
                else:
                    nc.vector.tensor_scalar(qkT[m][:], ps[:], 1.0 / WS, None,
                                            ALU.mult)

            # ---- B: v token-major fp8, DoubleRow token-pair layout ----
            # vt_dr[k2][p, h, hidx*65+c] = v[token 256*k2+128*h+p, head
            # block], col 64 of each 65-block = 1/(1-g); h-stride padded
            # to 400 bytes (DoubleRow needs step%16==0).
            VW = 400
            vt_dr = [vpool.tile([128, 2 * VW], F8, tag=f"v{k2}", name=f"v{k2}")
                     for k2 in range(TC // 2)]

            def vtv(kc):
                return (vt_dr[kc // 2][:]
                        .rearrange("p (h x) -> p h x", x=VW)
                        [:, kc % 2:kc % 2 + 1, 0:HL * (D + 1)]
                        .rearrange("p o (hh c) -> p o hh c", c=D + 1))

            for kc in range(TC):
                nc.gpsimd.dma_start(vtv(kc)[:, :, :, D:D + 1], vcol_t[:])
            for kc in range(TC):
                pool = psV0 if kc % 2 == 0 else psV1
                psv = pool.tile([128, HL * D], F32,
                                tag="v0" if kc % 2 == 0 else "v1")
                for k in range(DK):
                    nc.tensor.matmul(
                        psv[:],
                        re2(xnT8_t[k][:])[:, :, kc * 128:(kc + 1) * 128],
                        re2(vw8_t[k][:]),
                        start=(k == 0), stop=(k == DK - 1),
                        perf_mode=DR,
                    )
                nc.vector.tensor_scalar(
                    vtv(kc)[:, :, :, 0:D],
                    psv[:].rearrange("p (h c) -> p h c", c=D),
                    1.0 / WS, None, ALU.mult,
                )

            # ---- C: positional branch -> Z (11 x 768), x64 scaled ----
            psm = psV0.tile([S, HL * D], F32, tag="v0")
            for kc in range(TC):
                nc.tensor.matmul(
                    psm[0:S, :],
                    oh_t[kc][:, 0:S],
                    vtv(kc)[:, :, :, 0:D],
                    start=(kc == 0), stop=(kc == TC - 1),
                )
            m1n = mpool.tile([S, HL * D], BF16, tag="m1n")
            nc.vector.tensor_tensor(m1n[0:S, :], psm[0:S, :], gscf_t[0:S, :],
                                    ALU.mult)
            pst = psV1.tile([128, 3 * 16], BF16, tag="v1")
            for k3 in range(PAIRS):
                nc.tensor.transpose(
                    pst[:, k3 * 16:k3 * 16 + S],
                    m1n[0:S, k3 * 128:(k3 + 1) * 128],
                    ident_t[0:S, 0:S],
                )
            m1T = mpool.tile([128, 3 * 16], BF16, tag="m1T")
            nc.vector.tensor_copy(m1T[:], pst[:])
            psz = psS.tile([S, C], F32, tag="s")
            for (c0, c1) in ((0, 512), (512, C)):
                for k3 in range(PAIRS):
                    nc.tensor.matmul(psz[0:S, c0:c1], m1T[:, k3 * 16:k3 * 16 + S],
                                     pw_t[k3][:, c0:c1],
                                     start=(k3 == 0), stop=(k3 == PAIRS - 1))
            zb = mpool.tile([S, C], BF16, tag="zb")
            nc.vector.tensor_scalar(zb[0:S, :], psz[0:S, :], WS, None, ALU.mult)

            # ---- D: attention, software-pipelined ----
            # pairs 0/1 normalise into the fp8 DoubleRow stationary
            # vcat01[(h=pair), j*64+d, token]; pair 2 into bf16 vcat2
            # scaled x64 via the PE broadcast.
            vcat01 = vcpool.tile([128, 2 * N], F8, tag="vc01")
            vcat2 = vcpool.tile([128, N], BF16, tag="vc2")

            def s_pair(p, kc):
                """QK^T for both heads of pair p, key chunk kc (row-split)."""
                t0 = psS.tile([128, N], F32, tag="s", name=f"sS{p}_{kc}_0")
                t1 = psS.tile([128, N], F32, tag="s", name=f"sS{p}_{kc}_1")
                for qc in range(QC):
                    for j, ps in ((0, t0), (1, t1)):
                        off = j * 64
                        nc.tensor.matmul(
                            ps[:, qc * 512:(qc + 1) * 512],
                            qkT[3 + p][off:off + 64, kc * 128:(kc + 1) * 128],
                            qkT[p][off:off + 64, qc * 512:(qc + 1) * 512],
                            start=True, stop=True,
                            tile_position=(off, 0),
                        )
                return t0, t1

            def pv_dr(p, j, k2, e_ap, psv):
                """P@V for one head over a 256-key pair chunk, fp8 DR."""
                hidx = 2 * p + j
                lhs = (vt_dr[k2][:].rearrange("p (h x) -> p h x", x=VW)
                       [:, :, hidx * (D + 1):(hidx + 1) * (D + 1)])
                rhs = e_ap.rearrange("p (h n) -> p h n", h=2)
                for qc in range(QC):
                    nc.tensor.matmul(
                        psv[0:D + 1, qc * 512:(qc + 1) * 512],
                        lhs,
                        rhs[:, :, qc * 512:(qc + 1) * 512],
                        start=(k2 == 0), stop=(k2 == TC // 2 - 1),
                        perf_mode=DR,
                    )

            def drain01(p, j, psv):
                """Pairs 0/1: reciprocal row -> gpsimd partition_broadcast
                (only Q7 op in the kernel, so a single library; its ~4us
                drain latency hides under the next pair) -> DVE multiply.
                A stride-0 DMA broadcast would be faster but its completion
                semaphore undercounts -> race -> intermittent NaN."""
                vcp = dpool.tile([65, N], BF16, tag=f"vcp{j}", name=f"vcp{p}_{j}")
                nc.scalar.copy(vcp[0:65, :], psv[0:65, :])
                packed = rpool.tile([64, 16], BF16, tag=f"packed{j}",
                                    name=f"packed{p}_{j}")
                nc.sync.dma_start(packed[:], vcp[64:65, :])
                rec = rpool.tile([64, 16], BF16, tag=f"rec{j}", name=f"rec{p}_{j}")
                with nc.allow_low_precision(reason="softmax denom, ample tol"):
                    nc.vector.reciprocal(rec[:], packed[:])
                rrt = rpool.tile([1, N], BF16, tag=f"rrt{j}", name=f"rrt{p}_{j}")
                nc.sync.dma_start(rrt[0:1, :], rec[:])
                rbc = dpool.tile([64, N], BF16, tag=f"rbc{j}", name=f"rbc{p}_{j}")
                nc.gpsimd.partition_broadcast(rbc[:], rrt[0:1, :])
                nc.vector.tensor_tensor(
                    re2(vcat01[:])[j * 64:(j + 1) * 64, p, :],
                    vcp[0:64, :], rbc[:], ALU.mult,
                )

            def drain2_pre(j, psv, rbc_pool, rbc_tag):
                """Pair 2, stage 1: copy + reciprocal row (through rrt).
                Allocates the broadcast PSUM tile now so later pool users
                order correctly; the PE matmul comes in drain2_mm."""
                vcp = dpool.tile([65, N], BF16, tag=f"vcp{j}", name=f"vcp2_{j}")
                if j == 0:
                    nc.scalar.copy(vcp[0:65, :], psv[0:65, :])
                else:
                    nc.vector.tensor_copy(vcp[0:65, :], psv[0:65, :])
                packed = rpool.tile([64, 16], BF16, tag=f"packed{j}",
                                    name=f"packed2_{j}")
                nc.sync.dma_start(packed[:], vcp[64:65, :])
                rec = rpool.tile([64, 16], BF16, tag=f"rec{j}", name=f"rec2_{j}")
                with nc.allow_low_precision(reason="softmax denom, ample tol"):
                    nc.vector.reciprocal(rec[:], packed[:])
                rrt = rpool.tile([1, N], BF16, tag=f"rrt{j}", name=f"rrt2_{j}")
                nc.sync.dma_start(rrt[0:1, :], rec[:])
                rbc_ps = rbc_pool.tile([64, N], F32, tag=rbc_tag,
                                       name=f"rbcps{j}")
                return vcp, rrt, rbc_ps

            def drain2_mm(j, vcp, rrt, rbc_ps):
                """Pair 2, stage 2: rank-1 PE broadcast (64/denom) then
                DVE multiply -> bf16 vcat2 (x64)."""
                for qc in range(QC):
                    nc.tensor.matmul(
                        rbc_ps[0:64, qc * 512:(qc + 1) * 512],
                        ones64[0:1, :],
                        rrt[0:1, qc * 512:(qc + 1) * 512],
                        start=True, stop=True,
                    )
                nc.vector.tensor_tensor(
                    vcat2[j * 64:(j + 1) * 64, :],
                    vcp[0:64, :], rbc_ps[0:64, :], ALU.mult,
                )

            pend = None
            for p in range(PAIRS):
                sA = s_pair(p, 0)
                psv0 = psV0.tile([D + 1, N], F32, tag="v0", name=f"psV0_{p}")
                psv1 = psV1.tile([D + 1, N], F32, tag="v1", name=f"psV1_{p}")
                prev = cur = None     # (e0d, e1d) of key-pair chunks
                for kc in range(TC):
                    k2, h = divmod(kc, 2)
                    if h == 0:
                        prev = cur
                        cur = (e0pool.tile([128, 2 * N], F8, tag="e0",
                                           name=f"e0_{p}_{k2}"),
                               e1pool.tile([128, 2 * N], U8, tag="e1",
                                           name=f"e1_{p}_{k2}"))
                    # exp j0 on ACT (exact, fp8 out), j1 on DVE (Schraudolph)
                    nc.scalar.activation(
                        cur[0][:].rearrange("p (h n) -> p h n", h=2)
                        [:, h:h + 1, :],
                        sA[0][:], AF.Exp, scale=SCALE)
                    with nc.allow_low_precision(reason="schraudolph exp"):
                        nc.vector.tensor_scalar(
                            cur[1][:, h * N:(h + 1) * N], sA[1][:],
                            A_SCH, B_SCH, ALU.mult, ALU.add
                        )
                    # PE block: [PV(lagged pair-chunk)] after [S(kc+1)]
                    if pend is not None:
                        pp, ppsv, pe1 = pend
                        pv_dr(pp, 1, TC // 2 - 1, pe1[:].bitcast(F8), ppsv)
                        drain01(pp, 1, ppsv)
                        pend = None
                    if kc < TC - 1:
                        sA = s_pair(p, kc + 1)
                    if prev is not None:
                        if h == 0:
                            pv_dr(p, 0, k2 - 1, prev[0][:], psv0)
                        else:
                            pv_dr(p, 1, k2 - 1, prev[1][:].bitcast(F8), psv1)
                pv_dr(p, 0, TC // 2 - 1, cur[0][:], psv0)
                if p < PAIRS - 1:
                    drain01(p, 0, psv0)
                    pend = (p, psv1, cur[1])
                else:
                    pv_dr(p, 1, TC // 2 - 1, cur[1][:].bitcast(F8), psv1)
                    pair2_psvs = (psv0, psv1)

            # ---- E: proj (+Z); pair-2 matmuls deferred 3 slots ----
            # (pair-2 drains are emitted after two e_partial blocks so the
            # PE chews on them while the reciprocal chain runs)
            po_tiles = {}

            def e_partial(t_i):
                pool, tag = ((psS, "s"), (psS, "s"), (psV0, "v0"),
                             (psV1, "v1"))[t_i % 4]
                po = pool.tile([128, C], F32, tag=tag, name=f"po{t_i}")
                po_tiles[t_i] = po
                for (c0, c1) in ((0, 512), (512, C)):
                    nc.tensor.matmul(
                        po[:, c0:c1],
                        oht_t[0:S, t_i * 128:(t_i + 1) * 128],
                        zb[0:S, c0:c1],
                        start=True, stop=False,
                    )
                    nc.tensor.matmul(
                        po[:, c0:c1],
                        re2(vcat01[:])[:, :, t_i * 128:(t_i + 1) * 128],
                        re2(pw8_t[:])[:, :, c0:c1],
                        start=False, stop=False,
                        perf_mode=DR,
                    )

            def e_final(t_i):
                po = po_tiles.pop(t_i)
                for (c0, c1) in ((0, 512), (512, C)):
                    nc.tensor.matmul(
                        po[:, c0:c1],
                        vcat2[:, t_i * 128:(t_i + 1) * 128],
                        pw_t[2][:, c0:c1],
                        start=False, stop=True,
                    )
                ot = opool.tile([128, C], BF16, tag="ot")
                if t_i % 2 == 0:
                    nc.scalar.mul(ot[:], po[:], 1.0 / WS)
                else:
                    nc.vector.tensor_scalar(ot[:], po[:], 1.0 / WS, None,
                                            ALU.mult)
                nc.sync.dma_start(out.ap()[t_i * 128:(t_i + 1) * 128, :], ot[:])

            e_partial(0)
            e_partial(1)
            pre0 = drain2_pre(0, pair2_psvs[0], psV0, "v0")
            pre1 = drain2_pre(1, pair2_psvs[1], psV1, "v1")
            drain2_mm(0, *pre0)
            drain2_mm(1, *pre1)
            e_partial(2)
            e_partial(3)
            for t_i in range(4, TC):
                e_final(t_i - 4)
                e_partial(t_i)
            for t_i in range(4, TC):
                e_final(t_i)

    nc.compile()
    return nc


def _sigmoid(x):
    return 1.0 / (1.0 + np.exp(-x))


def _pack_dr(m):
    """[R, C] -> [R/2, 2C]: row 128k+p holds rows 256k+p | 256k+128+p."""
    r, c = m.shape
    return (m.reshape(r // 256, 2, 128, c).transpose(0, 2, 1, 3)
            .reshape(r // 2, 2 * c))


def _prep_core_inputs(cid, x, sector_ids, qkv_w, proj_w, gate_logit,
                      norm1_w, norm1_b, ls1_gamma):
    b, hg = cid // 2, cid % 2
    bf = ml_dtypes.bfloat16
    f8 = ml_dtypes.float8_e4m3
    h0 = hg * HL

    xb = x[b].astype(np.float64)
    mu = xb.mean(axis=-1, keepdims=True)
    var = xb.var(axis=-1, keepdims=True)
    xn = ((xb - mu) / np.sqrt(var + EPS)) * norm1_w + norm1_b  # (N, C)

    wq = qkv_w[:, h0 * D:(h0 + HL) * D]
    wk = qkv_w[:, C + h0 * D:C + (h0 + HL) * D]
    wv = qkv_w[:, 2 * C + h0 * D:2 * C + (h0 + HL) * D]
    qkw = np.concatenate([wq, wk], axis=1)
    pw_eff = proj_w[h0 * D:(h0 + HL) * D, :] * ls1_gamma[None, :]  # (384,768)

    g = _sigmoid(gate_logit.astype(np.float64))[h0:h0 + HL]  # (6,)

    onehot = np.zeros((N, S), np.float32)
    onehot[np.arange(N), sector_ids] = 1.0
    counts = onehot.sum(axis=0)                              # (11,)
    gsc = (g[None, :] / np.maximum(counts, 1.0)[:, None])    # (11, 6)
    gscf = np.repeat(gsc, D, axis=1).astype(np.float32)      # (11, 384)
    vcol = np.broadcast_to((1.0 / (1.0 - g))[None, :], (128, HL))

    return {
        "xnT8": np.ascontiguousarray(_pack_dr(xn.T).astype(f8)),
        "qkw8": np.ascontiguousarray(_pack_dr(qkw * WS).astype(f8)),
        "vw8": np.ascontiguousarray(_pack_dr(wv * WS).astype(f8)),
        "pw8": np.ascontiguousarray(
            (pw_eff[0:256] * WS).reshape(2, 128, C).transpose(1, 0, 2)
            .reshape(128, 2 * C).astype(f8)),
        "pw": np.ascontiguousarray(pw_eff.astype(bf)),
        "oh": np.ascontiguousarray(onehot.astype(f8)),
        "oht": np.ascontiguousarray(onehot.T.astype(bf)),
        "gscf": gscf,
        "vcol": np.ascontiguousarray(vcol.astype(f8)),
    }


def kernel(x, sector_ids, qkv_w, proj_w, proj_b, gate_logit,
           norm1_w, norm1_b, ls1_gamma, norm2_w, norm2_b,
           ff_w1, ff_b1, ff_w2, ff_b2, _want_trace=False):
    x = np.asarray(x, np.float32)
    sector_ids = np.asarray(sector_ids).astype(np.int64)
    args = [np.asarray(a, np.float32) for a in
            (qkv_w, proj_w, gate_logit, norm1_w, norm1_b, ls1_gamma)]

    in_maps = [_prep_core_inputs(cid, x, sector_ids, *args) for cid in range(8)]

    if "prog" not in _CACHED:
        _CACHED["prog"] = _build_program()
    nc = _CACHED["prog"]

    import concourse.mybir as _mb
    expected = set()
    for alloc in nc.m.functions[0].allocations:
        if isinstance(alloc, _mb.MemoryLocationSet) and alloc.kind == "ExternalInput":
            expected.add(alloc.memorylocations[0].name)
    in_maps = [{k: v for k, v in m.items() if k in expected} for m in in_maps]

    res = bass_utils.run_bass_kernel_spmd(
        nc, in_maps, core_ids=list(range(8)), trace=_want_trace
    )
    if _want_trace:
        _CACHED["last_result"] = res

    outs = [r["out"].astype(np.float32) for r in res.results]
    proj_b = np.asarray(proj_b, np.float32)
    ls1 = np.asarray(ls1_gamma, np.float32)
    full = np.empty((B, N, C), np.float32)
    for b in range(B):
        full[b] = x[b] + outs[2 * b] + outs[2 * b + 1] + (ls1 * proj_b)[None, :]
    return full
